# revision 7
# baseline (speedup 1.0000x reference)
"""GNN neighbor-max kernel — bf16 ap_gather + pair-max-tree design.

Per core: 2 samples, batch-parallel across the 8 NeuronCores. Per sample:
  bf16 table xe[16g+q, m, j] = x[8q+j, (m + 512g) % N]  (one copy per GPSIMD
  group, rolled by the group's node base so each group's own nodes sit at
  uniform offsets; 64KB/partition, double-buffered across samples).
  Group g owns nodes [g*512, (g+1)*512); its per-chunk index list holds the
  16 neighbors of 32 nodes, pre-shifted by -512g mod N: one ap_gather of 512
  idx -> gt [128, 512*8] bf16.
  DVE reduces k=16 via a contiguous pair-max tree (16->8->4->2->1, 2-byte
  packed innermost so the DVE 2x mode applies), then a final max against the
  table's own x slice (self node, uniform offset thanks to the roll) writes
  transposed into oblk [128, (j, n)].
  Per-sample oblk buffers; scalar engine drains halves to out[C, N] bf16.
"""

import numpy as np
import ml_dtypes

import concourse.bacc as bacc
import concourse.bass as bass
import concourse.mybir as mybir
from concourse.bass_utils import run_bass_kernel_spmd

B, C, N, K = 16, 128, 4096, 16
N_CORES = 8
S = B // N_CORES
D = 8                      # channels per partition block
NG = 8                     # gpsimd groups
NPG = N // NG              # 512 nodes per group
CHUNKS = 16
NPC = NPG // CHUNKS        # 32 nodes per group per chunk
NI = NPC * K               # 512 idx per gather
NCOLS = NPG * K // 16      # 512 idx columns per sample per partition

_NC_CACHE = {}


def _build_program():
    nc = bacc.Bacc(None, target_bir_lowering=False)

    bf16 = mybir.dt.bfloat16
    xe_d = nc.dram_tensor("xe", [S, C, N * D], bf16, kind="ExternalInput")
    idx_d = nc.dram_tensor("idx", [S, C, NCOLS], mybir.dt.int16,
                           kind="ExternalInput")
    out_d = nc.dram_tensor("out", [S, C, N], bf16, kind="ExternalOutput")

    with (
        nc.Block() as block,
        nc.semaphore("isem") as isem,   # idx DMAs
        nc.semaphore("t0sem") as t0sem,  # table 0 DMA
        nc.semaphore("t1sem") as t1sem,  # table 1 DMA
        nc.semaphore("gsem") as gsem,   # gather chunks done
        nc.semaphore("bsem") as bsem,   # tree level A done (gt free)
        nc.semaphore("msem") as msem,   # per-half-sample maxes done
        nc.semaphore("osem") as osem,   # out DMAs done
        nc.sbuf_tensor("tbl0", [C, N * D], bf16) as tbl0,      # 64KB/p
        nc.sbuf_tensor("tbl1", [C, N * D], bf16) as tbl1,      # 64KB/p
        nc.sbuf_tensor("gt0", [C, NI * D], bf16) as gt0,       # 8KB/p
        nc.sbuf_tensor("gt1", [C, NI * D], bf16) as gt1,
        nc.sbuf_tensor("gt2", [C, NI * D], bf16) as gt2,
        nc.sbuf_tensor("gt3", [C, NI * D], bf16) as gt3,
        nc.sbuf_tensor("tA", [C, NPC * 8 * D], bf16) as tA,    # 4KB/p
        nc.sbuf_tensor("tB", [C, NPC * 4 * D], bf16) as tB,
        nc.sbuf_tensor("tC", [C, NPC * 2 * D], bf16) as tC,
        nc.sbuf_tensor("tD", [C, NPC * D], bf16) as tD,
        nc.sbuf_tensor("ob0", [C, D * NPG], bf16) as ob0,      # 8KB/p
        nc.sbuf_tensor("ob1", [C, D * NPG], bf16) as ob1,
        nc.sbuf_tensor("idxt", [C, S * NCOLS], mybir.dt.int16) as idxt,
    ):
        tbls = [tbl0, tbl1]
        gts = [gt0, gt1, gt2, gt3]
        obs = [ob0, ob1]

        @block.sync
        def _(sy: bass.BassEngine):
            # tables only — idx rides the scalar engine's queue in parallel
            tsems = [t0sem, t1sem]
            for s in range(S):
                sy.dma_start(out=tbls[s][:], in_=xe_d[s]).then_inc(tsems[s], 16)

        @block.gpsimd
        def _(g: bass.BassGpSimd):
            g.wait_ge(isem, 16 * S)
            for s in range(S):
                g.wait_ge([t0sem, t1sem][s], 16)
                for c in range(CHUNKS):
                    ci = s * CHUNKS + c
                    if ci >= 4:
                        g.wait_ge(bsem, ci - 3)
                    col0 = s * NCOLS + c * (NI // 16)
                    g.ap_gather(
                        out_ap=gts[ci % 4][:],
                        in_ap=tbls[s][:],
                        idxs_ap=idxt[:, col0:col0 + NI // 16],
                        channels=C, num_elems=N, d=D, num_idxs=NI,
                    ).then_inc(gsem, 1)

        @block.vector
        def _(v: bass.BassVectorEngine):
            for s in range(S):
                for c in range(CHUNKS):
                    ci = s * CHUNKS + c
                    v.wait_ge(gsem, ci + 1)
                    gv = gts[ci % 4][:].rearrange("p (n k j) -> p n k j",
                                                  k=K, j=D)
                    av = tA[:].rearrange("p (n t j) -> p n t j", t=8, j=D)
                    bv = tB[:].rearrange("p (n t j) -> p n t j", t=4, j=D)
                    cv = tC[:].rearrange("p (n t j) -> p n t j", t=2, j=D)
                    dv = tD[:].rearrange("p (n j) -> p n j", j=D)
                    v.tensor_max(out=av, in0=gv[:, :, 0:K:2, :],
                                 in1=gv[:, :, 1:K:2, :]).then_inc(bsem, 1)
                    v.tensor_max(out=bv, in0=av[:, :, 0:8:2, :],
                                 in1=av[:, :, 1:8:2, :])
                    v.tensor_max(out=cv, in0=bv[:, :, 0:4:2, :],
                                 in1=bv[:, :, 1:4:2, :])
                    v.tensor_max(out=dv, in0=cv[:, :, 0, :],
                                 in1=cv[:, :, 1, :])
                    sv = tbls[s][:].rearrange("p (n j) -> p n j", j=D)[
                        :, c * NPC:(c + 1) * NPC, :]
                    ov = obs[s][:].rearrange("p (j n) -> p j n", j=D)
                    ov = ov.transpose([0, 2, 1])[:, c * NPC:(c + 1) * NPC, :]
                    e = v.tensor_max(out=ov, in0=dv, in1=sv)
                    if c == CHUNKS // 2 - 1 or c == CHUNKS - 1:
                        e.then_inc(msem, 1)

        @block.scalar
        def _(sc: bass.BassEngine):
            for s in range(S):
                sc.dma_start(out=idxt[:, s * NCOLS:(s + 1) * NCOLS],
                             in_=idx_d[s]).then_inc(isem, 16)
            half = NPG // 2
            for s in range(S):
                for h in range(2):
                    sc.wait_ge(msem, 2 * s + h + 1)
                    for gg in range(NG):
                        src = obs[s][gg * 16:(gg + 1) * 16].rearrange(
                            "p (j n) -> p j n", j=D)[:, :, h * half:(h + 1) * half]
                        dst = bass.AP(
                            out_d,
                            s * C * N + gg * NPG + h * half,
                            [[D * N, 16], [N, D], [1, half]],
                        )
                        sc.dma_start(out=dst, in_=src).then_inc(osem, 16)

    nc.compile()
    return nc


def _prep_sample(x_s: np.ndarray, nidx_s: np.ndarray):
    """x_s [C, N] f32, nidx_s [N, K] int -> (xe [C, N*D] bf16, idx [C, NCOLS] i16)."""
    xq = x_s.reshape(16, D, N).transpose(0, 2, 1)          # [q, n, j]
    xe = np.empty((NG, 16, N, D), dtype=np.float32)
    for g in range(NG):
        xe[g] = np.roll(xq, -NPG * g, axis=1)              # group-rolled copy
    xe = xe.reshape(C, N * D).astype(ml_dtypes.bfloat16)
    nidx = np.asarray(nidx_s, dtype=np.int64)              # [N, K]
    blocks = []
    for g in range(NG):
        blk = (nidx[g * NPG:(g + 1) * NPG] - NPG * g) % N  # [512, 16]
        flat = blk.reshape(-1).astype(np.int16)            # node-major
        blocks.append(flat.reshape(-1, 16).T)              # [16, 512]
    idx = np.concatenate(blocks, axis=0)                   # [128, 512]
    return np.ascontiguousarray(xe), np.ascontiguousarray(idx)


def _run(x: np.ndarray, neighbor_idx: np.ndarray, **spmd_kwargs):
    x = np.asarray(x, dtype=np.float32)
    neighbor_idx = np.asarray(neighbor_idx)

    if "nc" not in _NC_CACHE:
        _NC_CACHE["nc"] = _build_program()
    nc = _NC_CACHE["nc"]

    in_maps = []
    for core in range(N_CORES):
        lo = core * S
        xes, idxs = [], []
        for s in range(S):
            xe, idx = _prep_sample(x[lo + s], neighbor_idx[lo + s])
            xes.append(xe)
            idxs.append(idx)
        in_maps.append({
            "xe": np.stack(xes, axis=0),
            "idx": np.stack(idxs, axis=0),
        })

    res = run_bass_kernel_spmd(nc, in_maps, core_ids=list(range(N_CORES)),
                               **spmd_kwargs)
    out = np.concatenate([res.results[core]["out"] for core in range(N_CORES)],
                         axis=0)
    return out.astype(np.float32), res


def kernel(x: np.ndarray, neighbor_idx: np.ndarray) -> np.ndarray:
    return _run(x, neighbor_idx)[0]


if __name__ == "__main__":
    rng = np.random.default_rng(0)
    xt = rng.standard_normal((B, C, N)).astype(np.float32)
    it = rng.integers(0, N, size=(B, N, K)).astype(np.int64)
    got = kernel(xt, it)
    ref = np.maximum(
        np.max(xt[np.arange(B)[:, None, None], :, it], axis=2).transpose(0, 2, 1),
        xt,
    )
    xb = xt.astype(ml_dtypes.bfloat16).astype(np.float32)
    refb = np.maximum(
        np.max(xb[np.arange(B)[:, None, None], :, it], axis=2).transpose(0, 2, 1),
        xb,
    )
    print("abs err vs f32 ref:", np.abs(got - ref).max())
    print("abs err vs bf16 ref:", np.abs(got - refb).max())
